# revision 15
# baseline (speedup 1.0000x reference)
"""CARAFE (content-aware reassembly of features) TRN2 Bass kernel.

Problem: input [8, 256, 64, 64], kernel_map [8, 100, 64, 64] (100 = up^2 *
k^2 with up=2, k=5), output [8, 256, 128, 128].

Strategy: data-parallel over batch across 8 NeuronCores (one image per
core).  Output rows are processed in pairs (h, h+1): the two rows' 25-tap
windows draw on exactly 6 input rows = 3 input row-pairs, so each
output-row-pair needs just 3 PSUM-accumulated matmuls of N=512 columns
(the two rows' (w, u) columns interleaved):

    out[c, n] += sum_p X2[p, c] * Band[p, n],   n = (w, hh, ui, uj)

X2 holds the input row-interleaved: partition p = 2*w' + rr carries
x[:, r + rr, w'], so any even-based row pair (r, r+1) is a single
[128, 128] access pattern (no duplicated copies of x in SBUF or HBM).

Band matrices hold kernel_map values on banded diagonals (w' = w + dj - 2)
and are built host-side in float8_e3m4 (the moving operand may be fp8
while the stationary x stays fp16; products accumulate in fp32 PSUM),
halving band HBM traffic.  PSUM eviction performs the pixel shuffle to
[c, 2h+ui, 2w+uj] and downcasts to fp16; output is upcast on the host.
"""

import numpy as np
import ml_dtypes

import concourse.bass as bass
import concourse.mybir as mybir
import concourse.tile as tile
from concourse.bass_utils import run_bass_kernel_spmd

B, C, H, W = 8, 256, 64, 64
K, UP, PAD = 5, 2, 2
U2 = UP * UP
H2, W2 = H * UP, W * UP
FP32 = mybir.dt.float32
FP16 = mybir.dt.float16
FP8 = mybir.dt.float8e3

NCOL2 = 2 * W * U2   # 512 matmul output columns per row-pair
HCH = 4              # output rows per band chunk (= 2 row-pairs)
NCH = H // HCH       # 16 chunks
NP = 6               # band slots per chunk (2 row-pairs x 3 input pairs)
TW = NP * NCOL2      # 3072: band chunk tile width
HQ = 4               # output rows per store DMA
GPC = 4              # g-columns (row pairs) per X2 tile
NXB = H // 2 // GPC  # 8 X2 tiles


def _build_bass():
    nc = bass.Bass()
    xe_d = nc.declare_dram_parameter("xe", [1, 128 * (H // 2) * C], FP16,
                                     isOutput=False)
    bp_d = nc.declare_dram_parameter("bp", [1, NCH * 128 * TW], FP8,
                                     isOutput=False)
    y_d = nc.declare_dram_parameter("y", [1, C * H2 * W2], FP16, isOutput=True)

    xe_v = xe_d[0].rearrange("(p f) -> p f", p=128)          # [128, 32*C]
    bp_v = bp_d[0].rearrange("(c p f) -> c p f", c=NCH, p=128)
    y_v = y_d[0].rearrange("(c h w) -> c h w", c=C, h=H2)    # [C, H2, W2]

    with tile.TileContext(nc) as tc:
        with (
            tc.tile_pool(name="xe", bufs=1) as xe_pool,
            tc.tile_pool(name="bp", bufs=3) as bp_pool,
            tc.tile_pool(name="stg", bufs=4) as stg_pool,
            tc.tile_pool(name="ps", bufs=6, space="PSUM") as ps_pool,
        ):
            xeb = [xe_pool.tile([128, GPC * C], FP16, name=f"xeb{i}", tag=f"xeb{i}")
                   for i in range(NXB)]
            n_loaded = 0

            def load_xe(i):
                nc.sync.dma_start(
                    xeb[i][:, :],
                    xe_v[:, i * GPC * C: (i + 1) * GPC * C],
                )

            bpts = {}

            def load_band(ci):
                bpts[ci] = bp_pool.tile([128, TW], FP8, name="bpt", tag="bpt")
                if ci == NCH - 1:   # slot 5 unused on the last chunk
                    nc.gpsimd.dma_start(bpts[ci][:, 0:5 * NCOL2],
                                        bp_v[ci][:, 0:5 * NCOL2])
                else:
                    nc.gpsimd.dma_start(bpts[ci][:, :], bp_v[ci])

            # fast start: chunk 0 in two pieces (slot 0 unused), x tile 0 in
            # two halves, interleaved so h=0's matmuls can begin early
            bpts[0] = bp_pool.tile([128, TW], FP8, name="bpt", tag="bpt")
            nc.scalar.dma_start(bpts[0][:, NCOL2:3 * NCOL2],
                                bp_v[0][:, NCOL2:3 * NCOL2])
            nc.sync.dma_start(xeb[0][:, 0:2 * C], xe_v[:, 0:2 * C])
            nc.gpsimd.dma_start(bpts[0][:, 3 * NCOL2:TW],
                                bp_v[0][:, 3 * NCOL2:TW])
            nc.sync.dma_start(xeb[0][:, 2 * C:GPC * C],
                              xe_v[:, 2 * C:GPC * C])
            n_loaded = 1

            n_band = 1
            for ci in range(NCH):
                # prefetch X2 tiles: chunk ci touches g in [2ci-1, 2ci+2]
                need = min(NXB, (2 * ci + 2) // GPC + 2)
                while n_loaded < need:
                    load_xe(n_loaded)
                    n_loaded += 1
                while n_band < min(NCH, ci + 3):
                    load_band(n_band)
                    n_band += 1

                bpt = bpts.pop(ci)

                stg = {}
                for lp in range(2):
                    h = 2 * (2 * ci + lp)          # even row of the pair
                    if h % HQ == 0:
                        for ch in range(2):
                            stg[ch] = stg_pool.tile([128, HQ, UP, W2], FP16,
                                                    name=f"stg{ch}", tag=f"stg{ch}")
                    js = [j for j in range(3)
                          if 0 <= h - 2 + 2 * j and h - 1 + 2 * j < H]
                    pss = []
                    for ch in range(2):
                        ps = ps_pool.tile([128, NCOL2], FP32)
                        pss.append(ps)
                        for t, j in enumerate(js):
                            r = h - 2 + 2 * j
                            g = r // 2
                            lhsT = xeb[g // GPC][:, (g % GPC) * C + ch * 128:
                                                 (g % GPC) * C + ch * 128 + 128]
                            k = lp * 3 + j
                            rhs = bpt[:, k * NCOL2: (k + 1) * NCOL2]
                            nc.tensor.matmul(
                                ps[:, :], lhsT, rhs,
                                start=(t == 0), stop=(t == len(js) - 1),
                            )
                    for ch in range(2):
                        psv = pss[ch][:, :].rearrange(
                            "p (w hh ui uj) -> p w hh ui uj", w=W, hh=2, ui=UP)
                        for hh in range(2):
                            src_t = psv[:, :, hh, :, :].transpose([0, 2, 1, 3])
                            dst = stg[ch][:, (h + hh) % HQ, :, :]
                            if ch == 0:
                                nc.vector.tensor_copy(dst, src_t)
                            else:
                                nc.scalar.copy(dst, src_t)
                    if ci == NCH - 1:
                        # tail: store each row-pair as soon as it is evicted
                        for ch in range(2):
                            nc.sync.dma_start(
                                y_v[ch * 128: ch * 128 + 128,
                                    UP * h: UP * (h + 2), :],
                                stg[ch][:, h % HQ: h % HQ + 2, :, :],
                            )
                    elif (h + 1) % HQ == HQ - 1:
                        hq0 = ((h + 1) // HQ) * HQ
                        for ch in range(2):
                            nc.sync.dma_start(
                                y_v[ch * 128: ch * 128 + 128,
                                    UP * hq0: UP * (hq0 + HQ), :],
                                stg[ch][:],
                            )
    _split_overfull_waits(nc)
    return nc


def _split_overfull_waits(nc):
    """Walrus caps sem-waits per instruction (1; 2 for EventSemaphore).
    Hoist excess waits onto inserted wait-only instructions."""
    n_new = 0
    for bb in nc.main_func.blocks:
        out, changed = [], False
        for ins in bb.instructions:
            si = ins.sync_info
            waits = list(si.on_wait) if (si is not None and si.on_wait) else []
            cap = 2 if isinstance(ins, mybir.InstEventSemaphore) else 1
            if len(waits) > cap:
                keep, extra = waits[-cap:], waits[:-cap]
                while extra:
                    chunk, extra = extra[:2], extra[2:]
                    n_new += 1
                    ev = mybir.InstEventSemaphore(
                        name=f"I-waitfix-{n_new}",
                        engine=ins.engine,
                        sync_info=mybir.SyncInfo(on_wait=chunk, on_update=[]),
                        ins=[],
                        outs=[],
                    )
                    nc.register_instruction(ev)
                    out.append(ev)
                ins.sync_info = mybir.SyncInfo(
                    on_wait=keep,
                    on_update=list(si.on_update) if si.on_update else [],
                )
                changed = True
            out.append(ins)
        if changed:
            bb.instructions = out
    return n_new


# --- host-side packing -----------------------------------------------------
# Global index arrays for the fat band chunk [128, NP*512]:
# col = 512*k + n,  n = 8w + 4hh + 2ui + uj,  slot k = 3*lp + j.
_PP = np.arange(128)[:, None]
_CC = np.arange(TW)[None, :]
_KS = _CC // NCOL2
_N = _CC % NCOL2
_W = _N >> 3
_HH = (_N >> 2) & 1
_U = _N & 3
_LPS = _KS // 3
_J = _KS % 3
_RR = _PP % 2
_WP = _PP // 2
_DJ = _WP + PAD - _W
_DI = 2 * _J + _RR - _HH
_VAL0 = (_DJ >= 0) & (_DJ < K) & (_DI >= 0) & (_DI < K)
_DJC = np.clip(_DJ, 0, K - 1)
_DIC = np.clip(_DI, 0, K - 1)


def _host_pack(x_b, km_b):
    """x_b: [C, H, W] fp32; km_b: [100, H, W] -> (xe fp16, bp fp8e3)."""
    # xe: [p, g, c] with p = 2*w' + rr -> x[c, 2g+rr, w']
    xe = (x_b.reshape(C, H // 2, 2, W)       # [c, g, rr, w']
              .transpose(3, 2, 1, 0)         # [w', rr, g, c]
              .reshape(128, (H // 2) * C)
              .astype(np.float16))
    km_r = km_b.reshape(U2, K * K, H, W)

    bp = np.zeros((NCH, 128, TW), np.float32)
    for ci in range(NCH):
        hkm = 2 * (2 * ci + _LPS) + _HH          # output row this tap serves
        r = 2 * (2 * ci + _LPS) - 2 + 2 * _J     # input pair base row
        valid = _VAL0 & (r >= 0) & (r + 1 < H)
        vals = km_r[_U, _DIC * K + _DJC, np.clip(hkm, 0, H - 1), _W]
        bp[ci] = np.where(valid, vals, 0.0)
    return (
        np.ascontiguousarray(xe.reshape(1, -1)),
        np.ascontiguousarray(
            bp.reshape(1, -1).astype(ml_dtypes.float8_e3m4)),
    )


_NC_CACHE = None


def _get_nc():
    global _NC_CACHE
    if _NC_CACHE is None:
        _NC_CACHE = _build_bass()
    return _NC_CACHE


def _prep_inputs(input, kernel_map):
    in_maps = []
    for b in range(B):
        xe, bp = _host_pack(np.asarray(input[b]), np.asarray(kernel_map[b]))
        in_maps.append({"xe": xe, "bp": bp})
    return in_maps


def _run(input, kernel_map, trace=False):
    nc = _get_nc()
    in_maps = _prep_inputs(input, kernel_map)
    res = run_bass_kernel_spmd(nc, in_maps, list(range(B)), trace=trace)
    out = np.stack(
        [res.results[b]["y"].reshape(C, H2, W2).astype(np.float32) for b in range(B)],
        axis=0,
    )
    return out, res


def kernel(input, kernel_map):
    out, _ = _run(input, kernel_map, trace=False)
    return out
